# revision 1
# baseline (speedup 1.0000x reference)
"""Multi-head self-attention (B=4, S=2048, D=2048, H=16, hd=128) on 8 trn2
NeuronCores.

Sharding: tensor-parallel over heads. Core c owns heads {2c, 2c+1}:
  - computes q/k/v projections for its 2 heads over all tokens,
  - runs causal attention for its (4 batches x 2 heads) units,
  - computes a partial output projection with its 256 rows of Wo.
Host sums the 8 partial outputs and adds bo.

On-chip layouts keep activations transposed ([feature, token]) so no
transposes are needed anywhere except V (PE-transpose per 128x128 block):
  qT/kT: [j, t] from W-stationary matmuls (lhsT = W tile, rhs = xT tile)
  S^T:   [t_k, t_q] tiles (lhsT = kT tile, rhs = qT chunk); softmax runs
         along the partition axis: exp on ACT (no max subtraction --
         weights are scaled 0.02, logits are O(1)); the exp-sum G
         accumulates in two partial chains (DVE + GPSIMD), is summed and
         broadcast across partitions by one GPSIMD partition_all_reduce,
         inverted in place (DVE reciprocal), and applied by one DVE
         multiply.
  U^T:   [j, t_q] = accumulated (lhsT = V tile [t_k, j], rhs = exp(S^T)).
  O^T:   [d', t] partial = (lhsT = Wo tile [j, d'], rhs = Yn^T).
Causality: only lower-triangle key-tiles are computed; the 4 diagonal
128x512 tile positions use precomputed 0/1 masks (multiplied after exp).

Scheduling shape (per batch): projections -> V transposes -> attention
units (h, c), with the previous unit's softmax normalization emitted at
the start of the next unit and the PREVIOUS batch's output-projection
tile-groups interleaved through the ACT-paced attention stream so the
tensor engine always has independent work.
"""

import math

import numpy as np
import ml_dtypes

import concourse.bass as bass
import concourse.bacc as bacc
import concourse.mybir as mybir
import concourse.tile as tile
from concourse.masks import make_identity
from concourse.bass_utils import run_bass_kernel_spmd

BF16 = mybir.dt.bfloat16
F32 = mybir.dt.float32

B, S, D_MODEL = 4, 2048, 2048
N_HEADS, HEAD_DIM = 16, 128
N_CORES = 8
H_PER = N_HEADS // N_CORES          # 2 heads per core
JL = H_PER * HEAD_DIM               # 256 local j-columns per of q/k/v
T = B * S                           # 8192 tokens
KD = D_MODEL // 128                 # 16 contraction tiles over d_model
TC = S // 512                       # 4 token chunks of 512 per batch
NJM = 3 * H_PER                     # 6 output j-tiles for fused qkv
SCALE = 1.0 / math.sqrt(HEAD_DIM)

_CACHED_NC = None
_OCOPY_MIX = True   # o_sb copies alternate DVE/ACT


def build_program():
    nc = bacc.Bacc("TRN2", target_bir_lowering=False, debug=False)

    xT = nc.dram_tensor("xT", [D_MODEL, T], BF16, kind="ExternalInput").ap()
    wqkv = nc.dram_tensor("wqkv", [D_MODEL, 3 * JL], BF16, kind="ExternalInput").ap()
    bqkv = nc.dram_tensor("bqkv", [3 * JL], F32, kind="ExternalInput").ap()
    wo = nc.dram_tensor("wo", [JL, D_MODEL], BF16, kind="ExternalInput").ap()
    outT = nc.dram_tensor("outT", [D_MODEL, T], F32, kind="ExternalOutput").ap()

    xT_r = xT.rearrange("(k p) t -> p k t", p=128)        # [128, KD, T]

    with tile.TileContext(nc) as tc:
        with (
            tc.tile_pool(name="const", bufs=1) as const,
            tc.tile_pool(name="work", bufs=1) as work,
            tc.tile_pool(name="psum", bufs=1, space="PSUM") as psum,
        ):
            # ---- constants ----
            # Load order matters at startup: the first matmul group only
            # needs wqkv k-chunk 0 and the first xt chunk (emitted by the
            # first _emit_batch), so everything else trails them.
            wqkv_sb = const.tile([128, KD, 3 * JL], BF16)
            wqkv_r = wqkv.rearrange("(k p) j -> p k j", p=128)
            nc.sync.dma_start(wqkv_sb[:, 0:4, :], wqkv_r[:, 0:4, :])
            bqkv_sb = const.tile([128, NJM], F32)
            nc.sync.dma_start(bqkv_sb[:], bqkv.rearrange("(m p) -> p m", p=128))

            def load_trailing_consts():
                for kc in range(1, 4):
                    nc.sync.dma_start(wqkv_sb[:, 4 * kc:4 * (kc + 1), :],
                                      wqkv_r[:, 4 * kc:4 * (kc + 1), :])
                nc.sync.dma_start(wo_sb[:], wo.rearrange("(k p) d -> p k d", p=128))
            wo_sb = const.tile([128, JL // 128, D_MODEL], BF16)

            ident = const.tile([128, 128], BF16)
            make_identity(nc, ident[:])
            ones_c = const.tile([128, 1], F32)
            nc.gpsimd.memset(ones_c[:], 1.0)

            # masks[i][r, u] = 1.0 if u >= 128*i + r else 0  (diagonal tiles)
            masks = const.tile([128, 4, 512], BF16)
            nc.gpsimd.memset(masks[:], 1.0)
            for i in range(4):
                nc.gpsimd.affine_select(
                    out=masks[:, i, :],
                    in_=masks[:, i, :],
                    compare_op=mybir.AluOpType.is_ge,
                    fill=0.0,
                    base=-128 * i,
                    pattern=[[1, 512]],
                    channel_multiplier=-1,
                )

            outproj_q = []       # deferred outproj thunks from previous batch
            for b in range(B):
                _emit_batch(nc, tc, work, psum, b,
                            xT_r, wqkv_sb, bqkv_sb, wo_sb,
                            ident, ones_c, masks, outT, outproj_q,
                            post_first_xt=load_trailing_consts if b == 0 else None)
            for thunk in outproj_q:
                thunk()

    nc.compile()
    return nc


def _emit_batch(nc, tc, work, psum, b, xT_r, wqkv_sb, bqkv_sb, wo_sb,
                ident, ones_c, masks, outT, outproj_q, post_first_xt=None):
    t0 = b * S

    # ---- q/k/v projections: qkvT[j, t] for the 6 local j-tiles ----
    qkvT = work.tile([128, NJM, S], BF16, tag="qkvT", bufs=2)
    for tcn in range(TC):
      with nc.named_scope(f"proj.b{b}.t{tcn}"):
        xt = work.tile([128, KD, 512], BF16, tag="xt", bufs=2)
        nc.sync.dma_start(xt[:], xT_r[:, :, t0 + tcn * 512: t0 + (tcn + 1) * 512])
        if post_first_xt is not None:
            post_first_xt()
            post_first_xt = None
        for jm in range(NJM):
            ps = psum.tile([128, 512], F32, tag="pp", bufs=2)
            for k in range(KD):
                nc.tensor.matmul(
                    ps[:],
                    lhsT=wqkv_sb[:, k, jm * 128:(jm + 1) * 128],
                    rhs=xt[:, k, :],
                    start=(k == 0), stop=(k == KD - 1),
                )
            nc.vector.tensor_scalar_add(
                qkvT[:, jm, tcn * 512:(tcn + 1) * 512], ps[:],
                bqkv_sb[:, jm:jm + 1],
            )

    # ---- V[t, j] per head via PE transpose of vT ----
    v_sb = work.tile([128, H_PER, S // 128, 128], BF16, tag="v", bufs=1)
    for h in range(H_PER):
      with nc.named_scope(f"vtr.b{b}.h{h}"):
        for m in range(S // 128):
            vt_ps = psum.tile([128, 128], BF16, tag="pp", bufs=2)
            nc.tensor.transpose(
                vt_ps[:], qkvT[:, 2 * H_PER + h, m * 128:(m + 1) * 128], ident[:]
            )
            nc.vector.tensor_copy(v_sb[:, h, m, :], vt_ps[:])

    # ---- attention ----
    yn = work.tile([128, H_PER, S], BF16, tag="yn", bufs=2)
    pending = None

    def emit_norm(p):
      with nc.named_scope(f"norm.b{b}"):
        gs_, u_, h_, c_ = p
        if len(gs_) > 1:
            nc.vector.tensor_add(gs_[0][:], gs_[0][:], gs_[1][:])
        import concourse.bass_isa as bass_isa
        rb_sb = work.tile([128, 512], F32, tag="rb", bufs=2)
        nc.gpsimd.partition_all_reduce(rb_sb[:], gs_[0][:], channels=128,
                                       reduce_op=bass_isa.ReduceOp.add)
        nc.vector.reciprocal(rb_sb[:], rb_sb[:])
        nc.vector.tensor_mul(
            yn[:, h_, c_ * 512:(c_ + 1) * 512], u_[:], rb_sb[:]
        )

    n_units = H_PER * TC
    per_unit = (len(outproj_q) + n_units - 1) // n_units if outproj_q else 0
    unit_idx = 0
    for h in range(H_PER):
        qT = qkvT[:, h, :]
        kT = qkvT[:, H_PER + h, :]
        for c in range(TC):
            nm = 4 * (c + 1)            # valid 128-wide key tiles
            # normalization of the previous unit goes first so its pool/DVE
            # ops are not stuck behind this unit's accumulation chain
            if pending is not None:
                emit_norm(pending)
                pending = None
            # The exp-sum G is accumulated in two independent partial chains
            # (DVE 2/3 of pairs, GPSIMD 1/3) so neither engine's serial chain
            # outlasts the unit; the norm that consumes them is deferred by
            # one unit, and sums both partials into one PSUM accumulator.
            with nc.named_scope(f"att.b{b}.u{unit_idx}"):
              g_d = g_p = None
              e_pairs = []
              npr = nm // 2
              # spread this unit's share of deferred outproj groups through
              # the ACT-paced pair loop so PE always has independent work
              spots = set(np.linspace(0, npr - 1, min(per_unit, npr)).astype(int).tolist()) if outproj_q else set()
              popped = 0
              for pr in range(nm // 2):
                  if pr in spots and outproj_q:
                      outproj_q.pop(0)()
                      popped += 1
                  s2 = psum.tile([128, 2, 512], F32, tag="s2", bufs=2)
                  for i in range(2):
                      m = 2 * pr + i
                      nc.tensor.matmul(
                          s2[:, i, :],
                          lhsT=kT[:, m * 128:(m + 1) * 128],
                          rhs=qT[:, c * 512:(c + 1) * 512],
                          start=True, stop=True,
                      )
                  e = work.tile([128, 2, 512], BF16, tag="e", bufs=9)
                  nc.scalar.activation(e[:], s2[:], mybir.ActivationFunctionType.Exp,
                                       scale=SCALE)
                  if pr >= nm // 2 - 2:   # diagonal pairs get the causal mask
                      i0 = 2 * (pr - (nm // 2 - 2))
                      nc.vector.tensor_mul(e[:], e[:], masks[:, i0:i0 + 2, :])
                  if pr % 3 == 2:
                      if g_p is None:
                          g_p = work.tile([128, 512], F32, tag="gp", bufs=2)
                          nc.gpsimd.tensor_add(g_p[:], e[:, 0, :], e[:, 1, :])
                      else:
                          nc.gpsimd.tensor_add(g_p[:], g_p[:], e[:, 0, :])
                          nc.gpsimd.tensor_add(g_p[:], g_p[:], e[:, 1, :])
                  else:
                      if g_d is None:
                          g_d = work.tile([128, 512], F32, tag="g", bufs=2)
                          nc.vector.tensor_add(g_d[:], e[:, 0, :], e[:, 1, :])
                      else:
                          nc.vector.tensor_add(g_d[:], g_d[:], e[:, 0, :])
                          nc.vector.tensor_add(g_d[:], g_d[:], e[:, 1, :])
                  e_pairs.append(e)

              u = psum.tile([128, 512], F32, tag="u", bufs=2)
              for m in range(nm):
                  nc.tensor.matmul(
                      u[:],
                      lhsT=v_sb[:, h, m, :],
                      rhs=e_pairs[m // 2][:, m % 2, :],
                      start=(m == 0), stop=(m == nm - 1),
                  )
              pending = ([g for g in (g_d, g_p) if g is not None], u, h, c)
            # remainder of this unit's share of deferred outproj groups
            for _ in range(per_unit - popped):
                if outproj_q:
                    outproj_q.pop(0)()
            unit_idx += 1
    emit_norm(pending)
    while outproj_q:
        outproj_q.pop(0)()

    # ---- partial output projection (deferred into the next batch) ----
    def make_outproj(dm, tcn, yn=yn, t0=t0):
        def thunk():
          with nc.named_scope(f"oproj.b{b}"):
            ps = psum.tile([128, 512], F32, tag="pp", bufs=2)
            for kj in range(JL // 128):
                nc.tensor.matmul(
                    ps[:],
                    lhsT=wo_sb[:, kj, dm * 128:(dm + 1) * 128],
                    rhs=yn[:, kj, tcn * 512:(tcn + 1) * 512],
                    start=(kj == 0), stop=(kj == JL // 128 - 1),
                )
            o_sb = work.tile([128, 512], F32, tag="osb", bufs=3)
            if _OCOPY_MIX and (dm * TC + tcn) % 2 == 1:
                nc.scalar.copy(o_sb[:], ps[:])
            else:
                nc.vector.tensor_copy(o_sb[:], ps[:])
            nc.sync.dma_start(
                outT[dm * 128:(dm + 1) * 128,
                     t0 + tcn * 512: t0 + (tcn + 1) * 512],
                o_sb[:],
            )
        return thunk

    for dm in range(D_MODEL // 128):
        for tcn in range(TC):
            outproj_q.append(make_outproj(dm, tcn))


def make_in_maps(x, Wq, bq, Wk, bk, Wv, bv, Wo, bo):
    xT_np = np.ascontiguousarray(
        x.reshape(T, D_MODEL).T).astype(ml_dtypes.bfloat16)
    in_maps = []
    for c in range(N_CORES):
        sl = slice(c * JL, (c + 1) * JL)
        wqkv_np = np.concatenate(
            [Wq[:, sl], Wk[:, sl], Wv[:, sl]], axis=1).astype(ml_dtypes.bfloat16)
        bqkv_np = np.concatenate([bq[sl], bk[sl], bv[sl]]).astype(np.float32)
        wo_np = np.ascontiguousarray(Wo[sl, :]).astype(ml_dtypes.bfloat16)
        in_maps.append({
            "xT": xT_np, "wqkv": wqkv_np, "bqkv": bqkv_np, "wo": wo_np,
        })
    return in_maps


def kernel(x, Wq, bq, Wk, bk, Wv, bv, Wo, bo):
    global _CACHED_NC
    x, Wq, bq, Wk, bk, Wv, bv, Wo, bo = [
        np.asarray(a, np.float32) for a in (x, Wq, bq, Wk, bk, Wv, bv, Wo, bo)
    ]
    if _CACHED_NC is None:
        _CACHED_NC = build_program()
    nc = _CACHED_NC

    in_maps = make_in_maps(x, Wq, bq, Wk, bk, Wv, bv, Wo, bo)
    res = run_bass_kernel_spmd(nc, in_maps, core_ids=list(range(N_CORES)))

    acc = res.results[0]["outT"].astype(np.float32)
    for c in range(1, N_CORES):
        acc += res.results[c]["outT"]
    out = acc.T + bo[None, :]
    return np.ascontiguousarray(out.reshape(B, S, D_MODEL), dtype=np.float32)


# ---------------------------------------------------------------- dev tools

def _np_partial_reference(inputs, core):
    """fp32 numpy partial output for one core's heads (no bo)."""
    x = np.asarray(inputs["x"], np.float32).reshape(T, D_MODEL)
    sl = slice(core * JL, (core + 1) * JL)
    q = x @ np.asarray(inputs["Wq"])[:, sl] + np.asarray(inputs["bq"])[sl]
    k = x @ np.asarray(inputs["Wk"])[:, sl] + np.asarray(inputs["bk"])[sl]
    v = x @ np.asarray(inputs["Wv"])[:, sl] + np.asarray(inputs["bv"])[sl]
    y = np.zeros((T, JL), np.float32)
    for b in range(B):
        tb = slice(b * S, (b + 1) * S)
        for h in range(H_PER):
            js = slice(h * HEAD_DIM, (h + 1) * HEAD_DIM)
            qh, kh, vh = q[tb, js], k[tb, js], v[tb, js]
            s = (qh @ kh.T) * SCALE
            mask = np.triu(np.ones((S, S), bool), k=1)
            s[mask] = -np.inf
            s -= s.max(axis=1, keepdims=True)
            p = np.exp(s)
            p /= p.sum(axis=1, keepdims=True)
            y[tb, js] = p @ vh
    return (y @ np.asarray(inputs["Wo"])[sl, :]).T  # [D, T]


def _simulate_core0():
    import reference
    from concourse.bass_interp import CoreSim

    inputs = {k: np.asarray(v) for k, v in reference.setup_inputs().items()}
    nc = build_program()
    in_map = make_in_maps(**inputs)[0]

    sim = CoreSim(nc)
    for name, arr in in_map.items():
        sim.tensor(name)[:] = arr
    sim.simulate(check_with_hw=False)
    got = np.asarray(sim.tensor("outT"), np.float32)

    want = _np_partial_reference(inputs, 0)
    denom = np.abs(want).max()
    err = np.abs(got - want).max() / denom
    print(f"sim core0 partial: max={np.abs(got).max():.4f} "
          f"absmax_err={np.abs(got - want).max():.5f} rel={err:.5f}")


if __name__ == "__main__":
    import sys
    if "--sim" in sys.argv:
        _simulate_core0()
    else:
        nc = build_program()
        n_inst = sum(len(bb.instructions) for bb in nc.m.functions[0].blocks)
        print(f"built: {n_inst} instructions")



# revision 39
# speedup vs baseline: 1.5704x; 1.5704x over previous
"""Multi-head self-attention (B=4, S=2048, D=2048, H=16, hd=128) on 8 trn2
NeuronCores.

Sharding: tensor-parallel over heads. Core c owns heads {2c, 2c+1}:
  - computes q/k/v projections for its 2 heads over all tokens,
  - runs causal attention for its (4 batches x 2 heads) units,
  - computes a partial output projection with its 256 rows of Wo.
Host sums the 8 partial outputs and adds bo.

On-chip layouts keep activations transposed ([feature, token]) so no
transposes are needed anywhere except V (PE-transpose per 128x128 block):
  qT/kT: [j, t] from W-stationary matmuls (lhsT = W tile, rhs = xT tile)
  S^T:   [t_k, t_q] tiles (lhsT = kT tile, rhs = qT chunk); softmax runs
         along the partition axis: exp on ACT (no max subtraction --
         weights are scaled 0.02, logits are O(1)); the exp-sum G
         accumulates in two partial chains (DVE + GPSIMD), is summed and
         broadcast across partitions by one GPSIMD partition_all_reduce,
         inverted in place (DVE reciprocal), and applied by one DVE
         multiply.
  U^T:   [j, t_q] = accumulated (lhsT = V tile [t_k, j], rhs = exp(S^T)).
  O^T:   [d', t] partial = (lhsT = Wo tile [j, d'], rhs = Yn^T).
Causality: only lower-triangle key-tiles are computed; the 4 diagonal
128x512 tile positions use precomputed 0/1 masks (multiplied after exp).

Scheduling shape (per batch): projections -> V transposes -> attention
units (h, c), with the previous unit's softmax normalization emitted at
the start of the next unit and the PREVIOUS batch's output-projection
tile-groups interleaved through the ACT-paced attention stream so the
tensor engine always has independent work.
"""

import math

import numpy as np
import ml_dtypes

import concourse.bass as bass
import concourse.bacc as bacc
import concourse.mybir as mybir
import concourse.tile as tile
from concourse.masks import make_identity
from concourse.bass_utils import run_bass_kernel_spmd

BF16 = mybir.dt.bfloat16
F16 = mybir.dt.float16
F32 = mybir.dt.float32
F8 = mybir.dt.float8e4

B, S, D_MODEL = 4, 2048, 2048
N_HEADS, HEAD_DIM = 16, 128
N_CORES = 8
H_PER = N_HEADS // N_CORES          # 2 heads per core
JL = H_PER * HEAD_DIM               # 256 local j-columns per of q/k/v
T = B * S                           # 8192 tokens
KD = D_MODEL // 128                 # 16 contraction tiles over d_model
TC = S // 512                       # 4 token chunks of 512 per batch
NJM = 3 * H_PER                     # 6 output j-tiles for fused qkv
SCALE = 1.0 / math.sqrt(HEAD_DIM)

# split-fp8 projection: x is pre-scaled by SX and split into hi+lo fp8
# k-slots; w by SW. q/k/v come out scaled by SX*SW = 256; scores carry
# 256^2, folded into the exp scale; outT carries 256, divided on host.
SX = 8.0
SW = 32.0
PSCALE = SX * SW                    # 256
NDR = 3 * KD // 2                   # 24 DoubleRow matmuls per proj tile
# constant attenuation folded into exp so the fp16 running sums G can't
# overflow (exp(s) can reach ~6e4); cancels in the softmax normalization
EBIAS = math.log(1.0 / 256.0)

_CACHED_NC = None
_OCOPY_MIX = True   # o_sb copies alternate DVE/ACT


class _FillerQ:
    """Two priority classes of self-contained PE work: small (outproj
    groups, ~0.4us) and big (projection groups, ~2.5us). Pops follow a
    small-small-big pattern so at most ~2 small drains are ever pending
    between big groups, keeping the 3-deep pp PSUM pipeline covered."""

    def __init__(self):
        self.small = []
        self.big = []
        self._cnt = 0

    def __len__(self):
        return len(self.small) + len(self.big)

    def pop(self):
        use_big = self._cnt % 3 == 2
        self._cnt += 1
        if use_big:
            q = self.big if self.big else self.small
        else:
            q = self.small if self.small else self.big
        if q:
            q.pop(0)()


def build_program():
    nc = bacc.Bacc("TRN2", target_bir_lowering=False, debug=False)

    xT = nc.dram_tensor("xT", [2 * D_MODEL, T], F8, kind="ExternalInput").ap()
    wqkv = nc.dram_tensor("wqkv", [2 * NDR * 128, 3 * JL], F8, kind="ExternalInput").ap()
    bqkv = nc.dram_tensor("bqkv", [3 * JL], F32, kind="ExternalInput").ap()
    wo = nc.dram_tensor("wo", [JL, D_MODEL], BF16, kind="ExternalInput").ap()
    outT = nc.dram_tensor("outT", [D_MODEL, T], BF16, kind="ExternalOutput").ap()

    xT_r = xT.rearrange("(k p) t -> p k t", p=128)        # [128, 2*KD, T]

    with tile.TileContext(nc) as tc:
        with (
            tc.tile_pool(name="const", bufs=1) as const,
            tc.tile_pool(name="work", bufs=1) as work,
            tc.tile_pool(name="psum", bufs=1, space="PSUM") as psum,
        ):
            # ---- constants ----
            # Load order matters at startup: the first matmul group only
            # needs wqkv i-chunk 0 and the first xt chunk (emitted by the
            # first _emit_batch), so everything else trails them.
            wqkv_sb = const.tile([128, NDR, 2, 3 * JL], F8)
            wqkv_r = wqkv.rearrange("(i g p) j -> p i g j", p=128, g=2)
            nc.sync.dma_start(wqkv_sb[:, 0:3, :, :], wqkv_r[:, 0:3, :, :])
            bqkv_sb = const.tile([128, NJM], F32)
            nc.sync.dma_start(bqkv_sb[:], bqkv.rearrange("(m p) -> p m", p=128))

            def load_trailing_consts():
                for kc in range(1, 8):
                    nc.sync.dma_start(wqkv_sb[:, 3 * kc:3 * (kc + 1), :, :],
                                      wqkv_r[:, 3 * kc:3 * (kc + 1), :, :])
                nc.sync.dma_start(wo_sb[:], wo.rearrange("(k p) d -> p k d", p=128))
            wo_sb = const.tile([128, JL // 128, D_MODEL], BF16)

            ident = const.tile([128, 128], BF16)
            make_identity(nc, ident[:])
            ones_c = const.tile([128, 1], F32)
            nc.gpsimd.memset(ones_c[:], 1.0)
            ebias_c = const.tile([128, 1], F32)
            nc.gpsimd.memset(ebias_c[:], EBIAS)

            # masks[i][r, u] = 1.0 if u >= 128*i + r else 0  (diagonal tiles)
            masks = const.tile([128, 4, 512], BF16)
            nc.gpsimd.memset(masks[:], 1.0)
            for i in range(4):
                nc.gpsimd.affine_select(
                    out=masks[:, i, :],
                    in_=masks[:, i, :],
                    compare_op=mybir.AluOpType.is_ge,
                    fill=0.0,
                    base=-128 * i,
                    pattern=[[1, 512]],
                    channel_multiplier=-1,
                )

            filler_q = _FillerQ()
            qkvT_all = {}        # per-batch qkvT tiles (created one batch early)
            v_sb_all = {}
            for b in range(B):
                _emit_batch(nc, tc, work, psum, b,
                            xT_r, wqkv_sb, bqkv_sb, wo_sb,
                            ident, ebias_c, masks, outT, filler_q, qkvT_all,
                            v_sb_all,
                            post_first_xt=load_trailing_consts if b == 0 else None)
            while len(filler_q):
                filler_q.pop()

    nc.compile()
    return nc


def _make_proj_groups(nc, work, psum, b, xT_r, wqkv_sb, bqkv_sb, qkvT, v_sb,
                      ident, split_first_dma=False):
    """Per-(tcn, jm) projection groups as self-contained filler thunks.

    Split-fp8 DoubleRow: per k-pair m (k-tiles 2m, 2m+1; x slots 4m..4m+3
    = xh0,xl0,xh1,xl1) three DR matmuls contract both k-tiles with hi*hi,
    hi*lo and lo*hi cross terms (w packed to match on the host).
    Each chunk's group list also carries the V transposes for the chunk's
    token range, placed right after the v-column projections they read.
    """
    t0 = b * S

    def make_dma(tcn, xt):
        def thunk():
            if split_first_dma and tcn == 0:
                for q4 in range(4):
                    nc.sync.dma_start(
                        xt[:, 8 * q4:8 * (q4 + 1), :],
                        xT_r[:, 8 * q4:8 * (q4 + 1), t0:t0 + 512])
            else:
                nc.sync.dma_start(
                    xt[:], xT_r[:, :, t0 + tcn * 512: t0 + (tcn + 1) * 512])
        return thunk

    def make_group(tcn, jm, xt):
        def thunk():
          with nc.named_scope(f"proj.b{b}.t{tcn}"):
            js = slice(jm * 128, (jm + 1) * 128)
            ps = psum.tile([128, 512], F32, tag="pp", bufs=3)
            for m in range(KD // 2):
                rhs_abc = (
                    xt[:, 4 * m:4 * m + 2, :],          # (xh0, xl0)
                    xt[:, 4 * m:4 * m + 3:2, :],        # (xh0, xh1)
                    xt[:, 4 * m + 2:4 * m + 4, :],      # (xh1, xl1)
                )
                for c3 in range(3):
                    i = 3 * m + c3
                    nc.tensor.matmul(
                        ps[:],
                        lhsT=wqkv_sb[:, i, :, js],
                        rhs=rhs_abc[c3],
                        start=(i == 0), stop=(i == NDR - 1),
                        perf_mode=mybir.MatmulPerfMode.DoubleRow,
                    )
            # GPSIMD can't read PSUM, so drains alternate DVE / ACT
            if jm % 2 == 0:
                nc.vector.tensor_scalar_add(
                    qkvT[:, jm, tcn * 512:(tcn + 1) * 512], ps[:],
                    bqkv_sb[:, jm:jm + 1],
                )
            else:
                nc.scalar.activation(
                    qkvT[:, jm, tcn * 512:(tcn + 1) * 512], ps[:],
                    mybir.ActivationFunctionType.Identity,
                    bias=bqkv_sb[:, jm:jm + 1],
                )
        return thunk

    def make_vtr(tcn, h):
        def thunk():
          with nc.named_scope(f"vtr.b{b}.h{h}"):
            for m in range(4 * tcn, 4 * tcn + 4):
                vt_ps = psum.tile([128, 128], BF16, tag="pp", bufs=3)
                nc.tensor.transpose(
                    vt_ps[:], qkvT[:, 2 * H_PER + h, m * 128:(m + 1) * 128],
                    ident[:]
                )
                nc.vector.tensor_copy(v_sb[:, h, m, :], vt_ps[:])
        return thunk

    chunks = []
    for tcn in range(TC):
        xt = work.tile([128, 2 * KD, 512], F8, tag="xt", bufs=2, name=f"xt{b}_{tcn}")
        dma = make_dma(tcn, xt)
        groups = [make_group(tcn, jm, xt) for jm in range(5)]
        groups.append(make_vtr(tcn, 0))
        groups.append(make_group(tcn, 5, xt))
        groups.append(make_vtr(tcn, 1))
        chunks.append((dma, groups))
    return chunks


def _emit_batch(nc, tc, work, psum, b, xT_r, wqkv_sb, bqkv_sb, wo_sb,
                ident, ebias_c, masks, outT, filler_q, qkvT_all, v_sb_all,
                post_first_xt=None):
    t0 = b * S

    # ---- q/k/v projections + V transposes ----
    # Batch 0's run inline (nothing to overlap with); later batches' were
    # already emitted as fillers during the previous batch's attention —
    # flush any stragglers now (attention below reads qkvT and v_sb).
    if b == 0:
        qkvT_all[0] = work.tile([128, NJM, S], BF16, tag="qkvT", bufs=2, name="qkvT0")
        v_sb_all[0] = work.tile([128, H_PER, S // 128, 128], BF16, tag="v",
                                bufs=2, name="v0")
        chunks = _make_proj_groups(nc, work, psum, 0, xT_r, wqkv_sb, bqkv_sb,
                                   qkvT_all[0], v_sb_all[0], ident,
                                   split_first_dma=True)
        for tcn, (dma, groups) in enumerate(chunks):
            dma()
            if tcn == 0:
                post_first_xt()
            for g in groups:
                g()
    qkvT = qkvT_all[b]
    v_sb = v_sb_all[b]
    while filler_q.big:
        filler_q.pop()

    # next batch's projection groups become fillers for this batch's
    # attention; chunk tcn is enqueued at unit boundary 2*tcn so its xt DMA
    # has a full chunk of lead time
    next_chunks = None
    if b + 1 < B:
        qkvT_all[b + 1] = work.tile([128, NJM, S], BF16, tag="qkvT", bufs=2,
                                    name=f"qkvT{b + 1}")
        v_sb_all[b + 1] = work.tile([128, H_PER, S // 128, 128], BF16, tag="v",
                                    bufs=2, name=f"v{b + 1}")
        next_chunks = _make_proj_groups(nc, work, psum, b + 1, xT_r,
                                        wqkv_sb, bqkv_sb, qkvT_all[b + 1],
                                        v_sb_all[b + 1], ident)

    # ---- attention (c-major unit order) ----
    yn = work.tile([128, H_PER, S], BF16, tag="yn", bufs=2)
    pending = None

    def emit_norm(p):
      with nc.named_scope(f"norm.b{b}"):
        gs_, u_, h_, c_ = p
        if len(gs_) > 1:
            nc.vector.tensor_add(gs_[0][:], gs_[0][:], gs_[1][:])
        import concourse.bass_isa as bass_isa
        rb_sb = work.tile([128, 512], F32, tag="rb", bufs=2)
        nc.gpsimd.partition_all_reduce(rb_sb[:], gs_[0][:], channels=128,
                                       reduce_op=bass_isa.ReduceOp.add)
        nc.vector.reciprocal(rb_sb[:], rb_sb[:])
        nc.vector.tensor_mul(
            yn[:, h_, c_ * 512:(c_ + 1) * 512], u_[:], rb_sb[:]
        )

    def make_outproj(dm, tcn, yn=yn, t0=t0, tag="pp"):
        def thunk():
          with nc.named_scope(f"oproj.b{b}"):
            ps = psum.tile([128, 512], F32, tag=tag, bufs=3)
            for kj in range(JL // 128):
                nc.tensor.matmul(
                    ps[:],
                    lhsT=wo_sb[:, kj, dm * 128:(dm + 1) * 128],
                    rhs=yn[:, kj, tcn * 512:(tcn + 1) * 512],
                    start=(kj == 0), stop=(kj == JL // 128 - 1),
                )
            o_sb = work.tile([128, 512], BF16, tag="osb", bufs=3)
            if _OCOPY_MIX and (dm * TC + tcn) % 2 == 1:
                nc.scalar.copy(o_sb[:], ps[:])
            else:
                nc.vector.tensor_copy(o_sb[:], ps[:])
            nc.sync.dma_start(
                outT[dm * 128:(dm + 1) * 128,
                     t0 + tcn * 512: t0 + (tcn + 1) * 512],
                o_sb[:],
            )
        return thunk

    n_units = H_PER * TC
    unit_idx = 0
    for c in range(TC):
        nm = 4 * (c + 1)                # valid 128-wide key tiles
        for h in range(H_PER):
            qT = qkvT[:, h, :]
            kT = qkvT[:, H_PER + h, :]
            # normalization of the previous unit goes first so its pool/DVE
            # ops are not stuck behind this unit's accumulation chain
            if pending is not None:
                emit_norm(pending)
                pending = None
            # enqueue work that just became ready at this boundary
            if unit_idx % 2 == 0:
                if next_chunks is not None and unit_idx // 2 < TC:
                    dma, groups = next_chunks[unit_idx // 2]
                    dma()
                    filler_q.big.extend(groups)
                if unit_idx >= 2:
                    cr = unit_idx // 2 - 1   # norms of (·, cr) now emitted
                    filler_q.small.extend(make_outproj(dm, cr)
                                          for dm in range(D_MODEL // 128))
            # The exp-sum G is accumulated in two independent partial chains
            # (DVE 2/3 of tiles, GPSIMD 1/3) so neither engine's serial chain
            # outlasts the unit; the norm that consumes them is deferred by
            # one unit, and sums both partials into one PSUM accumulator.
            # Score tiles go diagonal-first so the masked tiles' exp+mask
            # are long done when the PV chain reaches them.
            with nc.named_scope(f"att.b{b}.u{unit_idx}"):
              g_d = g_p = None
              e_tiles = [None] * nm
              m_order = list(range(nm - 4, nm)) + list(range(nm - 4))
              # spread fillers through the ACT-paced loop so PE always has
              # independent work; aim to drain the queue by batch end
              units_left = n_units - unit_idx
              allow = min(len(filler_q), -(-len(filler_q) // units_left) + 2)
              spots = {}
              if allow:
                  for sp in np.linspace(0, nm - 1, allow).astype(int).tolist():
                      spots[sp] = spots.get(sp, 0) + 1
              for mi, m in enumerate(m_order):
                  for _ in range(spots.get(mi, 0)):
                      filler_q.pop()
                  s2 = psum.tile([128, 512], F32, tag="s2", bufs=3)
                  nc.tensor.matmul(
                      s2[:],
                      lhsT=kT[:, m * 128:(m + 1) * 128],
                      rhs=qT[:, c * 512:(c + 1) * 512],
                      start=True, stop=True,
                  )
                  e = work.tile([128, 512], BF16, tag="e", bufs=18)
                  nc.scalar.activation(e[:], s2[:], mybir.ActivationFunctionType.Exp,
                                       scale=SCALE / (PSCALE * PSCALE),
                                       bias=ebias_c[:])
                  if m >= nm - 4:        # diagonal tiles get the causal mask
                      nc.vector.tensor_mul(e[:], e[:], masks[:, m - (nm - 4), :])
                  if mi % 3 == 2:
                      if g_p is None:
                          g_p = work.tile([128, 512], F16, tag="gp", bufs=2)
                          nc.gpsimd.tensor_copy(g_p[:], e[:])
                      else:
                          nc.gpsimd.tensor_add(g_p[:], g_p[:], e[:])
                  else:
                      if g_d is None:
                          g_d = work.tile([128, 512], F16, tag="g", bufs=2)
                          nc.vector.tensor_copy(g_d[:], e[:])
                      else:
                          nc.vector.tensor_add(g_d[:], g_d[:], e[:])
                  e_tiles[m] = e

              u = psum.tile([128, 512], F32, tag="u", bufs=2)
              for m in range(nm):
                  nc.tensor.matmul(
                      u[:],
                      lhsT=v_sb[:, h, m, :],
                      rhs=e_tiles[m][:],
                      start=(m == 0), stop=(m == nm - 1),
                  )
              pending = ([g for g in (g_d, g_p) if g is not None], u, h, c)
            unit_idx += 1
    emit_norm(pending)
    # last chunk's outproj joins the queue; for the final batch the caller
    # flushes everything that remains, so those groups alternate PSUM tags
    # (s2 is idle by then) for a deeper drain pipeline in the bare tail
    tail = b == B - 1
    filler_q.small.extend(
        make_outproj(dm, TC - 1, tag=("s2" if tail and dm % 2 else "pp"))
        for dm in range(D_MODEL // 128))


def _split_f8(a):
    hi = a.astype(ml_dtypes.float8_e4m3)
    lo = (a - hi.astype(np.float32)).astype(ml_dtypes.float8_e4m3)
    return hi, lo


def make_in_maps(x, Wq, bq, Wk, bk, Wv, bv, Wo, bo):
    # x: scale by SX, split hi/lo fp8, interleave k-slots (xh0,xl0,xh1,xl1,..)
    xT_np = np.ascontiguousarray(x.reshape(T, D_MODEL).T) * np.float32(SX)
    xh, xl = _split_f8(xT_np)
    xdr = np.stack([xh.reshape(KD, 128, T), xl.reshape(KD, 128, T)],
                   axis=1).reshape(2 * D_MODEL, T)

    in_maps = []
    for c in range(N_CORES):
        sl = slice(c * JL, (c + 1) * JL)
        w = np.concatenate(
            [Wq[:, sl], Wk[:, sl], Wv[:, sl]], axis=1) * np.float32(SW)
        wh, wl = _split_f8(w)
        wh = wh.reshape(KD, 128, 3 * JL)
        wl = wl.reshape(KD, 128, 3 * JL)
        # DR instruction i groups (g=0,1); per k-pair m: A=(wh0,wh0),
        # B=(wl0,wh1), C=(wh1,wl1) matching the x-slot APs in _emit_batch
        wdr = np.empty((NDR, 2, 128, 3 * JL), ml_dtypes.float8_e4m3)
        for m in range(KD // 2):
            k0, k1 = 2 * m, 2 * m + 1
            wdr[3 * m, 0] = wh[k0]
            wdr[3 * m, 1] = wh[k0]
            wdr[3 * m + 1, 0] = wl[k0]
            wdr[3 * m + 1, 1] = wh[k1]
            wdr[3 * m + 2, 0] = wl[k1]
            wdr[3 * m + 2, 1] = wh[k1]
        bqkv_np = (np.concatenate([bq[sl], bk[sl], bv[sl]])
                   * np.float32(PSCALE)).astype(np.float32)
        wo_np = np.ascontiguousarray(Wo[sl, :]).astype(ml_dtypes.bfloat16)
        in_maps.append({
            "xT": xdr,
            "wqkv": wdr.reshape(2 * NDR * 128, 3 * JL),
            "bqkv": bqkv_np,
            "wo": wo_np,
        })
    return in_maps


def kernel(x, Wq, bq, Wk, bk, Wv, bv, Wo, bo):
    global _CACHED_NC
    x, Wq, bq, Wk, bk, Wv, bv, Wo, bo = [
        np.asarray(a, np.float32) for a in (x, Wq, bq, Wk, bk, Wv, bv, Wo, bo)
    ]
    if _CACHED_NC is None:
        _CACHED_NC = build_program()
    nc = _CACHED_NC

    in_maps = make_in_maps(x, Wq, bq, Wk, bk, Wv, bv, Wo, bo)
    res = run_bass_kernel_spmd(nc, in_maps, core_ids=list(range(N_CORES)))

    acc = res.results[0]["outT"].astype(np.float32)
    for c in range(1, N_CORES):
        acc += res.results[c]["outT"].astype(np.float32)
    out = acc.T * np.float32(1.0 / PSCALE) + bo[None, :]
    return np.ascontiguousarray(out.reshape(B, S, D_MODEL), dtype=np.float32)


# ---------------------------------------------------------------- dev tools

def _np_partial_reference(inputs, core):
    """fp32 numpy partial output for one core's heads (no bo)."""
    x = np.asarray(inputs["x"], np.float32).reshape(T, D_MODEL)
    sl = slice(core * JL, (core + 1) * JL)
    q = x @ np.asarray(inputs["Wq"])[:, sl] + np.asarray(inputs["bq"])[sl]
    k = x @ np.asarray(inputs["Wk"])[:, sl] + np.asarray(inputs["bk"])[sl]
    v = x @ np.asarray(inputs["Wv"])[:, sl] + np.asarray(inputs["bv"])[sl]
    y = np.zeros((T, JL), np.float32)
    for b in range(B):
        tb = slice(b * S, (b + 1) * S)
        for h in range(H_PER):
            js = slice(h * HEAD_DIM, (h + 1) * HEAD_DIM)
            qh, kh, vh = q[tb, js], k[tb, js], v[tb, js]
            s = (qh @ kh.T) * SCALE
            mask = np.triu(np.ones((S, S), bool), k=1)
            s[mask] = -np.inf
            s -= s.max(axis=1, keepdims=True)
            p = np.exp(s)
            p /= p.sum(axis=1, keepdims=True)
            y[tb, js] = p @ vh
    return (y @ np.asarray(inputs["Wo"])[sl, :]).T  # [D, T]


def _simulate_core0():
    import reference
    from concourse.bass_interp import CoreSim

    inputs = {k: np.asarray(v) for k, v in reference.setup_inputs().items()}
    nc = build_program()
    in_map = make_in_maps(**inputs)[0]

    sim = CoreSim(nc)
    for name, arr in in_map.items():
        sim.tensor(name)[:] = arr
    sim.simulate(check_with_hw=False)
    got = np.asarray(sim.tensor("outT"), np.float32) / PSCALE

    want = _np_partial_reference(inputs, 0)
    denom = np.abs(want).max()
    err = np.abs(got - want).max() / denom
    print(f"sim core0 partial: max={np.abs(got).max():.4f} "
          f"absmax_err={np.abs(got - want).max():.5f} rel={err:.5f}")


if __name__ == "__main__":
    import sys
    if "--sim" in sys.argv:
        _simulate_core0()
    else:
        nc = build_program()
        n_inst = sum(len(bb.instructions) for bb in nc.m.functions[0].blocks)
        print(f"built: {n_inst} instructions")

